# revision 26
# baseline (speedup 1.0000x reference)
"""Trainium2 Bass kernel for the batched CA_event ODE-RHS problem.

Computes, for B = 8388608 independent systems (per batch element):
    xn = (x/10)^2 ; yn = (y/10)^2 ; sn = 0.25
    hx = xn/(sn+xn) ; hy = yn/(sn+yn) ; rx = 1-hy ; ry = 1-hx
    u  = W0*(x+e_x-t0) + W1*(y+e_y-t1)
    dx = 10*(hx + 0.2*rx - 0.11*x + u*hx)
    dy = 10*(hy + 0.2*ry - 0.11*y)
    out = [dx, dy, -dx, -dy]            # shape [B, 4]

Rewritten in reciprocal form (R = 10*(1-h) = 2.5/(n+0.25) = 1/(0.004*s^2+0.1)):
    dx = (10-Rx)*(1+u) + 0.2*Ry - 1.1*x
    dy = (10-Ry) + 0.2*Rx - 1.1*y

Sharding: batch split evenly across 8 NeuronCores (trivially data parallel).
Per-core chunk of 1048576 elements is viewed as [128, 8192] (partition-major).
"""

import sys

import numpy as np

try:
    import concourse  # noqa: F401
except ImportError:  # pragma: no cover - fallback for bare environments
    sys.path.insert(0, "/opt/trn_rl_repo")

B = 8388608
N_CORES = 8
P = 128
BC = B // N_CORES          # 1048576 elements per core
COLS = BC // P             # 8192 free-dim columns per core
F = 1024                   # tile columns per loop iteration
N_IT = COLS // F

_COMPILED = {}


def _build(t0: float, t1: float, fast_recip: bool = False):
    """Trace + compile the per-core Tile kernel. Returns a ready Bass object."""
    from contextlib import ExitStack

    import concourse.bacc as bacc
    import concourse.tile as tile
    from concourse import mybir

    f32 = mybir.dt.float32
    ADD = mybir.AluOpType.add
    SUB = mybir.AluOpType.subtract
    MUL = mybir.AluOpType.mult
    SQUARE = mybir.ActivationFunctionType.Square
    COPY = mybir.ActivationFunctionType.Copy

    nc = bacc.Bacc("TRN2", target_bir_lowering=False, debug=False,
                   num_devices=N_CORES)

    xy_d = nc.dram_tensor("xy", [P, 2 * COLS], f32, kind="ExternalInput").ap()
    exy_d = nc.dram_tensor("exy", [P, 2 * COLS], f32,
                           kind="ExternalInput").ap()
    w_d = nc.dram_tensor("w", [P, 2 * COLS], f32, kind="ExternalInput").ap()
    o_d = nc.dram_tensor("out", [P, 4 * COLS], f32, kind="ExternalOutput").ap()

    with tile.TileContext(nc) as tc:
        with ExitStack() as ctx:
            io = ctx.enter_context(tc.tile_pool(name="io", bufs=2))
            tp = ctx.enter_context(tc.tile_pool(name="tmp", bufs=2))

            assert t0 == t1

            prev = None  # (ot, dxy, c, fsz) pending output assembly

            def emit_out(prev):
                # column-halved so each 2MB out-DMA can start as soon as its
                # half of the copies lands (shortens the kernel tail)
                ot, dxy, c, fsz = prev
                dxy3 = dxy.rearrange("p (f l) -> p f l", l=2)
                ot3 = ot.rearrange("p (f l) -> p f l", l=4)
                h = fsz // 2
                for k in range(2):
                    sl = slice(k * h, (k + 1) * h)
                    nc.scalar.activation(ot3[:, sl, 0:2], dxy3[:, sl], COPY)
                    nc.scalar.activation(ot3[:, sl, 2:4], dxy3[:, sl], COPY,
                                         scale=-1.0)
                    nc.sync.dma_start(
                        o_d[:, 4 * c + 4 * k * h:4 * c + 4 * (k + 1) * h],
                        ot[:, 4 * k * h:4 * (k + 1) * h])

            chunks = [(0, F // 2), (F // 2, F // 2)]
            chunks += [(i * F, F) for i in range(1, N_IT)]

            for c, fsz in chunks:
                xy = io.tile([P, 2 * fsz], f32, tag="xy", bufs=3)
                exy = io.tile([P, 2 * fsz], f32, tag="exy")
                wt = io.tile([P, 2 * fsz], f32, tag="w")
                ot = io.tile([P, 4 * fsz], f32, tag="out")

                # packed layout: block i0 holds [a-chunk-F | b-chunk-F]
                i0, off = c // F, c % F
                base = 2 * F * i0 + off
                if fsz == F:
                    nc.sync.dma_start(xy[:], xy_d[:, base:base + 2 * F])
                    nc.sync.dma_start(exy[:], exy_d[:, base:base + 2 * F])
                    nc.sync.dma_start(wt[:], w_d[:, base:base + 2 * F])
                else:
                    for tl, dr in ((xy, xy_d), (exy, exy_d), (wt, w_d)):
                        nc.sync.dma_start(tl[:, :fsz], dr[:, base:base + fsz])
                        nc.sync.dma_start(
                            tl[:, fsz:], dr[:, base + F:base + F + fsz])

                v = tp.tile([P, 2 * fsz], f32, tag="v")
                r = tp.tile([P, 2 * fsz], f32, tag="r", bufs=1)
                rx = tp.tile([P, fsz], f32, tag="rx", bufs=1)
                pq = tp.tile([P, 2 * fsz], f32, tag="pq")
                m = tp.tile([P, 2 * fsz], f32, tag="m")
                dxy = tp.tile([P, 2 * fsz], f32, tag="dxy")
                x11 = tp.tile([P, fsz], f32, tag="x11", bufs=1)
                y11n = tp.tile([P, fsz], f32, tag="y11n", bufs=1)
                u1 = tp.tile([P, fsz], f32, tag="u1", bufs=1)
                gn = tp.tile([P, fsz], f32, tag="gn", bufs=1)
                cx = tp.tile([P, fsz], f32, tag="cx", bufs=1)
                ty = tp.tile([P, fsz], f32, tag="ty", bufs=1)

                # control input path first: pq (DVE) -> m (GpSimd) so the
                # slow gpsimd multiply overlaps the DVE reciprocal chain
                nc.vector.scalar_tensor_tensor(pq[:], xy[:], -t0,
                                               exy[:], ADD, ADD)
                nc.gpsimd.tensor_mul(m[:], wt[:], pq[:])

                # Hill reciprocal terms: R = 2.5/(n+0.25) = 1/(0.4*n+0.1)
                # with n = (0.1*xy)^2 ; R = [Rx | Ry]
                nc.scalar.activation(v[:], xy[:], SQUARE, scale=0.1)
                nc.scalar.activation(v[:], v[:], COPY, scale=0.4, bias=0.1)
                nc.vector.reciprocal_approx_fast(out=r[:], in_=v[:])
                if not fast_recip:
                    # one Newton step on the x-half only: Rx's error is
                    # amplified by (1+u) downstream, Ry's is not
                    from concourse.dve_ops import RECIPROCAL_APPROX_NR
                    nc.vector._custom_dve(RECIPROCAL_APPROX_NR, out=rx[:],
                                          in0=v[:, :fsz], in1=r[:, :fsz],
                                          s0=2.0)

                # dy = (10-Ry) + 0.2*Rx - 1.1*y = ty - y11n
                nc.scalar.activation(y11n[:], xy[:, fsz:], COPY, scale=1.1,
                                     bias=-10.0)
                nc.vector.scalar_tensor_tensor(ty[:], rx[:], 0.2, r[:, fsz:],
                                               MUL, SUB)
                nc.vector.tensor_sub(dxy[:, 1::2], ty[:], y11n[:])

                # dx = (10-Rx)*u' + 0.2*Ry - 1.1*x   (gn = (Rx-10)*u' = -g)
                nc.scalar.activation(x11[:], xy[:, :fsz], COPY, scale=-1.1)
                nc.vector.scalar_tensor_tensor(cx[:], r[:, fsz:], 0.2, x11[:],
                                               MUL, ADD)
                nc.vector.scalar_tensor_tensor(u1[:], m[:, :fsz], 1.0,
                                               m[:, fsz:], ADD, ADD)
                nc.vector.scalar_tensor_tensor(gn[:], rx[:], 10.0, u1[:],
                                               SUB, MUL)
                nc.vector.tensor_sub(dxy[:, 0::2], cx[:], gn[:])

                # output assembly of the PREVIOUS chunk is emitted after this
                # one's compute so ACT prioritises the reciprocal chain
                if prev is not None:
                    emit_out(prev)
                prev = (ot, dxy, c, fsz)

            emit_out(prev)

    nc.compile()
    return nc


FAST_RECIP = False


def _get_nc(t0: float, t1: float):
    key = (t0, t1, FAST_RECIP)
    if key not in _COMPILED:
        _COMPILED[key] = _build(t0, t1, fast_recip=FAST_RECIP)
    return _COMPILED[key]


def run_sharded(x, y, e_x, e_y, W_a, target, trace=False, **run_kwargs):
    """Shard inputs over 8 cores, run the Bass kernel, gather full output.

    Returns (out[B,4] float32, BassKernelResults).
    """
    from concourse.bass_utils import run_bass_kernel_spmd

    x = np.ascontiguousarray(x, dtype=np.float32)
    y = np.ascontiguousarray(y, dtype=np.float32)
    e_x = np.ascontiguousarray(e_x, dtype=np.float32)
    e_y = np.ascontiguousarray(e_y, dtype=np.float32)
    W_a = np.ascontiguousarray(W_a, dtype=np.float32)
    target = np.asarray(target, dtype=np.float32)
    assert x.shape == (B,) and W_a.shape == (B, 2) and target.shape == (2,)

    t0, t1 = float(target[0]), float(target[1])
    nc = _get_nc(t0, t1)

    # Host-side packing: per-iteration blocks so each tile is ONE dma.
    #   xy[:, i, :]  = [x-chunk-i | y-chunk-i]
    #   exy[:, i, :] = [ex-chunk-i | ey-chunk-i]
    #   w[:, i, :]   = [W0-chunk-i | W1-chunk-i]
    def pack2(a, b):
        out = np.empty((N_CORES, P, N_IT, 2 * F), dtype=np.float32)
        out[:, :, :, :F] = a.reshape(N_CORES, P, N_IT, F)
        out[:, :, :, F:] = b.reshape(N_CORES, P, N_IT, F)
        return out.reshape(N_CORES, P, 2 * COLS)

    wv = W_a.reshape(N_CORES, P, N_IT, F, 2)
    wp = np.empty((N_CORES, P, N_IT, 2 * F), dtype=np.float32)
    wp[:, :, :, :F] = wv[..., 0]
    wp[:, :, :, F:] = wv[..., 1]
    wp = wp.reshape(N_CORES, P, 2 * COLS)
    xyp = pack2(x, y)
    exyp = pack2(e_x, e_y)

    in_maps = []
    for i in range(N_CORES):
        in_maps.append({
            "xy": xyp[i],
            "exy": exyp[i],
            "w": wp[i],
        })

    res = run_bass_kernel_spmd(nc, in_maps, list(range(N_CORES)),
                               trace=trace, **run_kwargs)
    out = np.empty((B, 4), dtype=np.float32)
    for i in range(N_CORES):
        out[i * BC:(i + 1) * BC] = res.results[i]["out"].reshape(BC, 4)
    return out, res


def kernel(x, y, e_x, e_y, W_a, target):
    out, _ = run_sharded(x, y, e_x, e_y, W_a, target)
    return out


# revision 27
# speedup vs baseline: 1.0218x; 1.0218x over previous
"""Trainium2 Bass kernel for the batched CA_event ODE-RHS problem.

Computes, for B = 8388608 independent systems (per batch element):
    xn = (x/10)^2 ; yn = (y/10)^2 ; sn = 0.25
    hx = xn/(sn+xn) ; hy = yn/(sn+yn) ; rx = 1-hy ; ry = 1-hx
    u  = W0*(x+e_x-t0) + W1*(y+e_y-t1)
    dx = 10*(hx + 0.2*rx - 0.11*x + u*hx)
    dy = 10*(hy + 0.2*ry - 0.11*y)
    out = [dx, dy, -dx, -dy]            # shape [B, 4]

Rewritten in reciprocal form (R = 10*(1-h) = 2.5/(n+0.25) = 1/(0.004*s^2+0.1)):
    dx = (10-Rx)*(1+u) + 0.2*Ry - 1.1*x
    dy = (10-Ry) + 0.2*Rx - 1.1*y

Sharding: batch split evenly across 8 NeuronCores (trivially data parallel).
Per-core chunk of 1048576 elements is viewed as [128, 8192] (partition-major).
"""

import sys

import numpy as np

try:
    import concourse  # noqa: F401
except ImportError:  # pragma: no cover - fallback for bare environments
    sys.path.insert(0, "/opt/trn_rl_repo")

B = 8388608
N_CORES = 8
P = 128
BC = B // N_CORES          # 1048576 elements per core
COLS = BC // P             # 8192 free-dim columns per core
F = 1024                   # tile columns per loop iteration
N_IT = COLS // F

_COMPILED = {}


def _build(t0: float, t1: float, fast_recip: bool = False):
    """Trace + compile the per-core Tile kernel. Returns a ready Bass object."""
    from contextlib import ExitStack

    import concourse.bacc as bacc
    import concourse.tile as tile
    from concourse import mybir

    f32 = mybir.dt.float32
    ADD = mybir.AluOpType.add
    SUB = mybir.AluOpType.subtract
    MUL = mybir.AluOpType.mult
    SQUARE = mybir.ActivationFunctionType.Square
    COPY = mybir.ActivationFunctionType.Copy

    nc = bacc.Bacc("TRN2", target_bir_lowering=False, debug=False,
                   num_devices=N_CORES)

    xy_d = nc.dram_tensor("xy", [P, 2 * COLS], f32, kind="ExternalInput").ap()
    exy_d = nc.dram_tensor("exy", [P, 2 * COLS], f32,
                           kind="ExternalInput").ap()
    w_d = nc.dram_tensor("w", [P, 2 * COLS], f32, kind="ExternalInput").ap()
    o_d = nc.dram_tensor("out", [P, 4 * COLS], f32, kind="ExternalOutput").ap()

    with tile.TileContext(nc) as tc:
        with ExitStack() as ctx:
            io = ctx.enter_context(tc.tile_pool(name="io", bufs=2))
            tp = ctx.enter_context(tc.tile_pool(name="tmp", bufs=2))

            assert t0 == t1

            prev = None  # (ot, dxy, c, fsz) pending output assembly

            def emit_out(prev):
                # column-halved so each 2MB out-DMA can start as soon as its
                # half of the copies lands (shortens the kernel tail)
                ot, dxy, c, fsz = prev
                dxy3 = dxy.rearrange("p (f l) -> p f l", l=2)
                ot3 = ot.rearrange("p (f l) -> p f l", l=4)
                h = fsz // 2
                for k in range(2):
                    sl = slice(k * h, (k + 1) * h)
                    nc.scalar.activation(ot3[:, sl, 0:2], dxy3[:, sl], COPY)
                    nc.scalar.activation(ot3[:, sl, 2:4], dxy3[:, sl], COPY,
                                         scale=-1.0)
                    nc.sync.dma_start(
                        o_d[:, 4 * c + 4 * k * h:4 * c + 4 * (k + 1) * h],
                        ot[:, 4 * k * h:4 * (k + 1) * h])

            chunks = [(i * F, F) for i in range(N_IT)]

            for c, fsz in chunks:
                xy = io.tile([P, 2 * fsz], f32, tag="xy", bufs=3)
                exy = io.tile([P, 2 * fsz], f32, tag="exy")
                wt = io.tile([P, 2 * fsz], f32, tag="w")
                ot = io.tile([P, 4 * fsz], f32, tag="out")

                # packed layout: block i0 holds [a-chunk-F | b-chunk-F]
                i0, off = c // F, c % F
                base = 2 * F * i0 + off
                if fsz == F:
                    nc.sync.dma_start(xy[:], xy_d[:, base:base + 2 * F])
                    nc.sync.dma_start(exy[:], exy_d[:, base:base + 2 * F])
                    nc.sync.dma_start(wt[:], w_d[:, base:base + 2 * F])
                else:
                    for tl, dr in ((xy, xy_d), (exy, exy_d), (wt, w_d)):
                        nc.sync.dma_start(tl[:, :fsz], dr[:, base:base + fsz])
                        nc.sync.dma_start(
                            tl[:, fsz:], dr[:, base + F:base + F + fsz])

                v = tp.tile([P, 2 * fsz], f32, tag="v")
                r = tp.tile([P, 2 * fsz], f32, tag="r", bufs=1)
                rx = tp.tile([P, fsz], f32, tag="rx", bufs=1)
                pq = tp.tile([P, 2 * fsz], f32, tag="pq")
                m = tp.tile([P, 2 * fsz], f32, tag="m")
                dxy = tp.tile([P, 2 * fsz], f32, tag="dxy")
                x11 = tp.tile([P, fsz], f32, tag="x11", bufs=1)
                y11n = tp.tile([P, fsz], f32, tag="y11n", bufs=1)
                u1 = tp.tile([P, fsz], f32, tag="u1", bufs=1)
                gn = tp.tile([P, fsz], f32, tag="gn", bufs=1)
                cx = tp.tile([P, fsz], f32, tag="cx", bufs=1)
                ty = tp.tile([P, fsz], f32, tag="ty", bufs=1)

                # control input path first: pq (DVE) -> m (GpSimd) so the
                # slow gpsimd multiply overlaps the DVE reciprocal chain
                nc.vector.scalar_tensor_tensor(pq[:], xy[:], -t0,
                                               exy[:], ADD, ADD)
                nc.gpsimd.tensor_mul(m[:], wt[:], pq[:])

                # Hill reciprocal terms: R = 2.5/(n+0.25) = 1/(0.4*n+0.1)
                # with n = (0.1*xy)^2 ; R = [Rx | Ry]
                nc.scalar.activation(v[:], xy[:], SQUARE, scale=0.1)
                nc.scalar.activation(v[:], v[:], COPY, scale=0.4, bias=0.1)
                nc.vector.reciprocal_approx_fast(out=r[:], in_=v[:])
                if not fast_recip:
                    # one Newton step on the x-half only: Rx's error is
                    # amplified by (1+u) downstream, Ry's is not
                    from concourse.dve_ops import RECIPROCAL_APPROX_NR
                    nc.vector._custom_dve(RECIPROCAL_APPROX_NR, out=rx[:],
                                          in0=v[:, :fsz], in1=r[:, :fsz],
                                          s0=2.0)

                # dy = (10-Ry) + 0.2*Rx - 1.1*y = ty - y11n
                nc.scalar.activation(y11n[:], xy[:, fsz:], COPY, scale=1.1,
                                     bias=-10.0)
                nc.vector.scalar_tensor_tensor(ty[:], rx[:], 0.2, r[:, fsz:],
                                               MUL, SUB)
                nc.vector.tensor_sub(dxy[:, 1::2], ty[:], y11n[:])

                # dx = (10-Rx)*u' + 0.2*Ry - 1.1*x   (gn = (Rx-10)*u' = -g)
                nc.scalar.activation(x11[:], xy[:, :fsz], COPY, scale=-1.1)
                nc.vector.scalar_tensor_tensor(cx[:], r[:, fsz:], 0.2, x11[:],
                                               MUL, ADD)
                nc.vector.scalar_tensor_tensor(u1[:], m[:, :fsz], 1.0,
                                               m[:, fsz:], ADD, ADD)
                nc.vector.scalar_tensor_tensor(gn[:], rx[:], 10.0, u1[:],
                                               SUB, MUL)
                nc.vector.tensor_sub(dxy[:, 0::2], cx[:], gn[:])

                # output assembly of the PREVIOUS chunk is emitted after this
                # one's compute so ACT prioritises the reciprocal chain
                if prev is not None:
                    emit_out(prev)
                prev = (ot, dxy, c, fsz)

            emit_out(prev)

    nc.compile()
    return nc


FAST_RECIP = False


def _get_nc(t0: float, t1: float):
    key = (t0, t1, FAST_RECIP)
    if key not in _COMPILED:
        _COMPILED[key] = _build(t0, t1, fast_recip=FAST_RECIP)
    return _COMPILED[key]


def run_sharded(x, y, e_x, e_y, W_a, target, trace=False, **run_kwargs):
    """Shard inputs over 8 cores, run the Bass kernel, gather full output.

    Returns (out[B,4] float32, BassKernelResults).
    """
    from concourse.bass_utils import run_bass_kernel_spmd

    x = np.ascontiguousarray(x, dtype=np.float32)
    y = np.ascontiguousarray(y, dtype=np.float32)
    e_x = np.ascontiguousarray(e_x, dtype=np.float32)
    e_y = np.ascontiguousarray(e_y, dtype=np.float32)
    W_a = np.ascontiguousarray(W_a, dtype=np.float32)
    target = np.asarray(target, dtype=np.float32)
    assert x.shape == (B,) and W_a.shape == (B, 2) and target.shape == (2,)

    t0, t1 = float(target[0]), float(target[1])
    nc = _get_nc(t0, t1)

    # Host-side packing: per-iteration blocks so each tile is ONE dma.
    #   xy[:, i, :]  = [x-chunk-i | y-chunk-i]
    #   exy[:, i, :] = [ex-chunk-i | ey-chunk-i]
    #   w[:, i, :]   = [W0-chunk-i | W1-chunk-i]
    def pack2(a, b):
        out = np.empty((N_CORES, P, N_IT, 2 * F), dtype=np.float32)
        out[:, :, :, :F] = a.reshape(N_CORES, P, N_IT, F)
        out[:, :, :, F:] = b.reshape(N_CORES, P, N_IT, F)
        return out.reshape(N_CORES, P, 2 * COLS)

    wv = W_a.reshape(N_CORES, P, N_IT, F, 2)
    wp = np.empty((N_CORES, P, N_IT, 2 * F), dtype=np.float32)
    wp[:, :, :, :F] = wv[..., 0]
    wp[:, :, :, F:] = wv[..., 1]
    wp = wp.reshape(N_CORES, P, 2 * COLS)
    xyp = pack2(x, y)
    exyp = pack2(e_x, e_y)

    in_maps = []
    for i in range(N_CORES):
        in_maps.append({
            "xy": xyp[i],
            "exy": exyp[i],
            "w": wp[i],
        })

    res = run_bass_kernel_spmd(nc, in_maps, list(range(N_CORES)),
                               trace=trace, **run_kwargs)
    out = np.empty((B, 4), dtype=np.float32)
    for i in range(N_CORES):
        out[i * BC:(i + 1) * BC] = res.results[i]["out"].reshape(BC, 4)
    return out, res


def kernel(x, y, e_x, e_y, W_a, target):
    out, _ = run_sharded(x, y, e_x, e_y, W_a, target)
    return out


# revision 28
# speedup vs baseline: 1.0618x; 1.0391x over previous
"""Trainium2 Bass kernel for the batched CA_event ODE-RHS problem.

Computes, for B = 8388608 independent systems (per batch element):
    xn = (x/10)^2 ; yn = (y/10)^2 ; sn = 0.25
    hx = xn/(sn+xn) ; hy = yn/(sn+yn) ; rx = 1-hy ; ry = 1-hx
    u  = W0*(x+e_x-t0) + W1*(y+e_y-t1)
    dx = 10*(hx + 0.2*rx - 0.11*x + u*hx)
    dy = 10*(hy + 0.2*ry - 0.11*y)
    out = [dx, dy, -dx, -dy]            # shape [B, 4]

Rewritten in reciprocal form (R = 10*(1-h) = 2.5/(n+0.25) = 1/(0.004*s^2+0.1)):
    dx = (10-Rx)*(1+u) + 0.2*Ry - 1.1*x
    dy = (10-Ry) + 0.2*Rx - 1.1*y

Sharding: batch split evenly across 8 NeuronCores (trivially data parallel).
Per-core chunk of 1048576 elements is viewed as [128, 8192] (partition-major).
"""

import sys

import numpy as np

try:
    import concourse  # noqa: F401
except ImportError:  # pragma: no cover - fallback for bare environments
    sys.path.insert(0, "/opt/trn_rl_repo")

B = 8388608
N_CORES = 8
P = 128
BC = B // N_CORES          # 1048576 elements per core
COLS = BC // P             # 8192 free-dim columns per core
F = 1024                   # tile columns per loop iteration
N_IT = COLS // F

_COMPILED = {}


def _build(t0: float, t1: float, fast_recip: bool = False):
    """Trace + compile the per-core Tile kernel. Returns a ready Bass object."""
    from contextlib import ExitStack

    import concourse.bacc as bacc
    import concourse.tile as tile
    from concourse import mybir

    f32 = mybir.dt.float32
    ADD = mybir.AluOpType.add
    SUB = mybir.AluOpType.subtract
    MUL = mybir.AluOpType.mult
    SQUARE = mybir.ActivationFunctionType.Square
    COPY = mybir.ActivationFunctionType.Copy

    nc = bacc.Bacc("TRN2", target_bir_lowering=False, debug=False,
                   num_devices=N_CORES)

    xy_d = nc.dram_tensor("xy", [P, 2 * COLS], f32, kind="ExternalInput").ap()
    exy_d = nc.dram_tensor("exy", [P, 2 * COLS], f32,
                           kind="ExternalInput").ap()
    w_d = nc.dram_tensor("w", [P, 2 * COLS], f32, kind="ExternalInput").ap()
    o_d = nc.dram_tensor("out", [P, 4 * COLS], f32, kind="ExternalOutput").ap()

    with tile.TileContext(nc) as tc:
        with ExitStack() as ctx:
            io = ctx.enter_context(tc.tile_pool(name="io", bufs=2))
            tp = ctx.enter_context(tc.tile_pool(name="tmp", bufs=2))

            assert t0 == t1

            prev = None  # (ot, dxy, c, fsz) pending output assembly

            def emit_out(prev):
                # column-halved so each 2MB out-DMA can start as soon as its
                # half of the copies lands (shortens the kernel tail)
                ot, dxy, c, fsz = prev
                dxy3 = dxy.rearrange("p (f l) -> p f l", l=2)
                ot3 = ot.rearrange("p (f l) -> p f l", l=4)
                h = fsz // 2
                for k in range(2):
                    sl = slice(k * h, (k + 1) * h)
                    nc.scalar.activation(ot3[:, sl, 0:2], dxy3[:, sl], COPY)
                    nc.scalar.activation(ot3[:, sl, 2:4], dxy3[:, sl], COPY,
                                         scale=-1.0)
                    nc.sync.dma_start(
                        o_d[:, 4 * c + 4 * k * h:4 * c + 4 * (k + 1) * h],
                        ot[:, 4 * k * h:4 * (k + 1) * h])

            chunks = [(i * F, F) for i in range(N_IT)]

            for c, fsz in chunks:
                xy = io.tile([P, 2 * fsz], f32, tag="xy", bufs=3)
                exy = io.tile([P, 2 * fsz], f32, tag="exy")
                wt = io.tile([P, 2 * fsz], f32, tag="w")
                ot = io.tile([P, 4 * fsz], f32, tag="out")

                # packed layout: block i0 holds [a-chunk-F | b-chunk-F]
                i0, off = c // F, c % F
                base = 2 * F * i0 + off
                if fsz == F:
                    nc.sync.dma_start(xy[:], xy_d[:, base:base + 2 * F])
                    nc.sync.dma_start(exy[:], exy_d[:, base:base + 2 * F])
                    nc.sync.dma_start(wt[:], w_d[:, base:base + 2 * F])
                else:
                    for tl, dr in ((xy, xy_d), (exy, exy_d), (wt, w_d)):
                        nc.sync.dma_start(tl[:, :fsz], dr[:, base:base + fsz])
                        nc.sync.dma_start(
                            tl[:, fsz:], dr[:, base + F:base + F + fsz])

                v = tp.tile([P, 2 * fsz], f32, tag="v")
                r = tp.tile([P, 2 * fsz], f32, tag="r", bufs=1)
                rx = (None if fast_recip else
                      tp.tile([P, fsz], f32, tag="rx", bufs=1))
                pq = tp.tile([P, 2 * fsz], f32, tag="pq")
                m = tp.tile([P, 2 * fsz], f32, tag="m")
                dxy = tp.tile([P, 2 * fsz], f32, tag="dxy")
                x11 = tp.tile([P, fsz], f32, tag="x11", bufs=1)
                y11n = tp.tile([P, fsz], f32, tag="y11n", bufs=1)
                u1 = tp.tile([P, fsz], f32, tag="u1", bufs=1)
                gn = tp.tile([P, fsz], f32, tag="gn", bufs=1)
                cx = tp.tile([P, fsz], f32, tag="cx", bufs=1)
                ty = tp.tile([P, fsz], f32, tag="ty", bufs=1)

                # control input path first: pq (DVE) -> m (GpSimd) so the
                # slow gpsimd multiply overlaps the DVE reciprocal chain
                nc.vector.scalar_tensor_tensor(pq[:], xy[:], -t0,
                                               exy[:], ADD, ADD)
                nc.gpsimd.tensor_mul(m[:], wt[:], pq[:])

                # Hill reciprocal terms: R = 2.5/(n+0.25) = 1/(0.4*n+0.1)
                # with n = (0.1*xy)^2 ; R = [Rx | Ry]
                nc.scalar.activation(v[:], xy[:], SQUARE, scale=0.1)
                nc.scalar.activation(v[:], v[:], COPY, scale=0.4, bias=0.1)
                nc.vector.reciprocal_approx_fast(out=r[:], in_=v[:])
                if fast_recip:
                    rxs = r[:, :fsz]
                else:
                    # one Newton step on the x-half only: Rx's error is
                    # amplified by (1+u) downstream, Ry's is not
                    from concourse.dve_ops import RECIPROCAL_APPROX_NR
                    nc.vector._custom_dve(RECIPROCAL_APPROX_NR, out=rx[:],
                                          in0=v[:, :fsz], in1=r[:, :fsz],
                                          s0=2.0)
                    rxs = rx[:]

                # dy = (10-Ry) + 0.2*Rx - 1.1*y = ty - y11n
                nc.scalar.activation(y11n[:], xy[:, fsz:], COPY, scale=1.1,
                                     bias=-10.0)
                nc.vector.scalar_tensor_tensor(ty[:], rxs, 0.2, r[:, fsz:],
                                               MUL, SUB)
                nc.vector.tensor_sub(dxy[:, 1::2], ty[:], y11n[:])

                # dx = (10-Rx)*u' + 0.2*Ry - 1.1*x   (gn = (Rx-10)*u' = -g)
                nc.scalar.activation(x11[:], xy[:, :fsz], COPY, scale=-1.1)
                nc.vector.scalar_tensor_tensor(cx[:], r[:, fsz:], 0.2, x11[:],
                                               MUL, ADD)
                nc.vector.scalar_tensor_tensor(u1[:], m[:, :fsz], 1.0,
                                               m[:, fsz:], ADD, ADD)
                nc.vector.scalar_tensor_tensor(gn[:], rxs, 10.0, u1[:],
                                               SUB, MUL)
                nc.vector.tensor_sub(dxy[:, 0::2], cx[:], gn[:])

                # output assembly of the PREVIOUS chunk is emitted after this
                # one's compute so ACT prioritises the reciprocal chain
                if prev is not None:
                    emit_out(prev)
                prev = (ot, dxy, c, fsz)

            emit_out(prev)

    nc.compile()
    return nc


FAST_RECIP = False


def _get_nc(t0: float, t1: float):
    key = (t0, t1, FAST_RECIP)
    if key not in _COMPILED:
        _COMPILED[key] = _build(t0, t1, fast_recip=FAST_RECIP)
    return _COMPILED[key]


def run_sharded(x, y, e_x, e_y, W_a, target, trace=False, **run_kwargs):
    """Shard inputs over 8 cores, run the Bass kernel, gather full output.

    Returns (out[B,4] float32, BassKernelResults).
    """
    from concourse.bass_utils import run_bass_kernel_spmd

    x = np.ascontiguousarray(x, dtype=np.float32)
    y = np.ascontiguousarray(y, dtype=np.float32)
    e_x = np.ascontiguousarray(e_x, dtype=np.float32)
    e_y = np.ascontiguousarray(e_y, dtype=np.float32)
    W_a = np.ascontiguousarray(W_a, dtype=np.float32)
    target = np.asarray(target, dtype=np.float32)
    assert x.shape == (B,) and W_a.shape == (B, 2) and target.shape == (2,)

    t0, t1 = float(target[0]), float(target[1])
    nc = _get_nc(t0, t1)

    # Host-side packing: per-iteration blocks so each tile is ONE dma.
    #   xy[:, i, :]  = [x-chunk-i | y-chunk-i]
    #   exy[:, i, :] = [ex-chunk-i | ey-chunk-i]
    #   w[:, i, :]   = [W0-chunk-i | W1-chunk-i]
    def pack2(a, b):
        out = np.empty((N_CORES, P, N_IT, 2 * F), dtype=np.float32)
        out[:, :, :, :F] = a.reshape(N_CORES, P, N_IT, F)
        out[:, :, :, F:] = b.reshape(N_CORES, P, N_IT, F)
        return out.reshape(N_CORES, P, 2 * COLS)

    wv = W_a.reshape(N_CORES, P, N_IT, F, 2)
    wp = np.empty((N_CORES, P, N_IT, 2 * F), dtype=np.float32)
    wp[:, :, :, :F] = wv[..., 0]
    wp[:, :, :, F:] = wv[..., 1]
    wp = wp.reshape(N_CORES, P, 2 * COLS)
    xyp = pack2(x, y)
    exyp = pack2(e_x, e_y)

    in_maps = []
    for i in range(N_CORES):
        in_maps.append({
            "xy": xyp[i],
            "exy": exyp[i],
            "w": wp[i],
        })

    res = run_bass_kernel_spmd(nc, in_maps, list(range(N_CORES)),
                               trace=trace, **run_kwargs)
    out = np.empty((B, 4), dtype=np.float32)
    for i in range(N_CORES):
        out[i * BC:(i + 1) * BC] = res.results[i]["out"].reshape(BC, 4)
    return out, res


def kernel(x, y, e_x, e_y, W_a, target):
    out, _ = run_sharded(x, y, e_x, e_y, W_a, target)
    return out
